# revision 12
# baseline (speedup 1.0000x reference)
"""Trainium2 Bass kernel for nn_PolymerGNN_SchNet_IV (gnn_message_passing).

Strategy (8 NeuronCores, SPMD — identical program, per-core data):
  - Atoms sharded by index range: core c owns atoms [c*2048, (c+1)*2048).
  - Edges sorted by dst on host; core c gets all edges whose dst it owns,
    grouped into 128-atom windows, padded to a uniform block count (BPW
    128-edge blocks per window) so every core runs the same NEFF.
  - Per interaction: x = h @ l1w computed on each core's atom shard,
    AllGather'ed into a full x-table in DRAM (runs on TOPSP+SDMA silicon,
    overlapped with the other molecule's compute). Messages gather x[src]
    via the custom dma_gather instruction; the segment-sum over dst becomes
    one-hot matmuls on the tensor engine accumulating in PSUM per window.
  - Edge filters W_i (i=0..5) are precomputed once per molecule into DRAM
    (bf16) and streamed back per interaction. All MLP biases are folded
    into augmented matmuls (softplus(0)=log2 trick for the filter MLP).
  - The per-graph readout collapses: mean over graphs of per-graph sums ==
    (sum over all atoms)/NGRAPHS, so batch vectors are irrelevant. Each
    core emits its [64] partial sums; the tiny fc head runs on host.
  - Perf: 4 SWDGE queues round-robin the dma_gathers (Q7 descriptor
    generation parallelizes ~4x across queue contexts); the AllGather
    output lives in Shared DRAM for the direct HBM-HBM collective path.
"""

import math
import numpy as np
import ml_dtypes

import concourse.bass as bass
import concourse.mybir as mybir
import concourse.tile as tile
from concourse import bacc, library_config
from concourse.bass_utils import run_bass_kernel_spmd
from concourse.masks import make_identity
import concourse.hw_specs as hw_specs

# Route every activation func to one shared table (natural_log_exp_and_others
# covers exp/ln/square/copy/identity/relu/abs) so the first-match table chooser
# doesn't alternate loads between exp_and_others and natural_log on every
# softplus (= Ln(Exp(x)+1)) pair. Table ids are list positions == json order,
# so only the advertised func sets are edited, never the order.
_orig_get_tables = hw_specs.get_activation_tables
_KEEP = {
    "natural_log_exp_and_others": None,           # keep everything
    "sqrt_and_others": {mybir.ActivationFunctionType.Sqrt},
    "trig_and_small": {mybir.ActivationFunctionType.Sin},
}


def _patched_tables(arch):
    d = _orig_get_tables(arch)
    out = {}
    for name, funcs in d.items():
        if name in _KEEP:
            out[name] = funcs if _KEEP[name] is None else _KEEP[name]
        else:
            out[name] = set()
    return out


hw_specs.get_activation_tables = _patched_tables
bacc.get_activation_tables = _patched_tables

F32 = mybir.dt.float32
BF16 = mybir.dt.bfloat16
I16 = mybir.dt.int16
F16 = mybir.dt.float16

LOG2 = 0.6931471805599453
ABLATE = set()  # debug: subset of {"wprod","gather_seq","no_oh","no_scatter"}
CUTOFF = 10.0
NGAUSS = 50
HID = 64
NINT = 6
NCORES = 8


class Cfg:
    def __init__(self, N, E, NGRAPHS):
        self.N = N
        self.E = E
        self.NGRAPHS = NGRAPHS
        self.APC = N // NCORES            # atoms per core
        assert self.APC % 128 == 0
        self.WPC = self.APC // 128        # windows per core
        self.NPAD = N + 8                 # x/pos table rows (row N.. are zero)


def _gather_layout(idx_flat):
    """[n*1024] int -> [128, n*64] int16 in dma_gather index layout."""
    a = np.asarray(idx_flat, dtype=np.int16).reshape(-1, 64, 16)
    a = a.transpose(2, 0, 1).reshape(16, -1)
    return np.ascontiguousarray(np.tile(a, (8, 1)))


def host_prep_mol(z, pos, edge, cfg):
    """Sort/shard/pad one molecule's edges. Returns per-core dict + shared."""
    N, APC, WPC = cfg.N, cfg.APC, cfg.WPC
    src = np.asarray(edge[0], dtype=np.int64)
    dst = np.asarray(edge[1], dtype=np.int64)
    order = np.argsort(dst, kind="stable")
    src_s = src[order]
    dst_s = dst[order]

    cores = []
    maxbpw = 0
    for c in range(NCORES):
        lo, hi = c * APC, (c + 1) * APC
        l = np.searchsorted(dst_s, lo)
        r = np.searchsorted(dst_s, hi)
        s_c, d_c = src_s[l:r], dst_s[l:r] - lo
        w_c = d_c >> 7
        cnt = np.bincount(w_c, minlength=WPC)
        maxbpw = max(maxbpw, int(np.ceil(cnt.max() / 128)))
        cores.append((s_c, d_c, w_c, cnt))
    return cores, maxbpw


def finish_prep_mol(cores, cfg, BPW):
    WPC = cfg.WPC
    NBLK = WPC * BPW
    out = []
    for (s_c, d_c, w_c, cnt) in cores:
        src_pad = np.full(NBLK * 128, cfg.N, dtype=np.int64)
        dst_pad = np.full(NBLK * 128, cfg.N, dtype=np.int64)  # dummy -> zero row
        rel_pad = np.zeros(NBLK * 128, dtype=np.float32)
        # window w occupies slots [w*BPW*128, ...)
        off = np.concatenate([[0], np.cumsum(cnt)])
        for w in range(WPC):
            seg = slice(off[w], off[w + 1])
            n = off[w + 1] - off[w]
            base = w * BPW * 128
            src_pad[base:base + n] = s_c[seg]
            dst_pad[base:base + n] = d_c[seg] + (np.arange(0) if False else 0)
            rel_pad[base:base + n] = (d_c[seg] - w * 128).astype(np.float32)
        # global dst index for pos gather (core-local dst + core base is added
        # by caller via d_c being core-local: reconstruct global below)
        out.append((src_pad, dst_pad, rel_pad, off))
    return out, NBLK


def prep_inputs(inputs, cfg):
    """Build per-core in_maps + shared meta. Returns (in_maps, meta)."""
    N, APC, WPC, NPAD = cfg.N, cfg.APC, cfg.WPC, cfg.NPAD
    mols = []
    maxbpw = 0
    for tag in ("A", "G"):
        z = np.asarray(inputs["z" + tag])
        pos = np.asarray(inputs["pos" + tag], dtype=np.float32)
        edge = np.asarray(inputs["edge" + tag])
        cores, mb = host_prep_mol(z, pos, edge, cfg)
        mols.append((tag, z, pos, edge, cores))
        maxbpw = max(maxbpw, mb)
    BPW = maxbpw
    while (WPC * BPW) % 8:
        BPW += 1
    NBLK = WPC * BPW

    # shared weights
    emb = np.asarray(inputs["emb"], dtype=np.float32)
    offset = np.linspace(0.0, CUTOFF, NGAUSS).astype(np.float32)
    coeff = float(-0.5 / (offset[1] - offset[0]) ** 2)

    mw1 = np.asarray(inputs["mlp_w1"], dtype=np.float32)
    mb1 = np.asarray(inputs["mlp_b1"], dtype=np.float32)
    mw2 = np.asarray(inputs["mlp_w2"], dtype=np.float32)
    mb2 = np.asarray(inputs["mlp_b2"], dtype=np.float32)
    l1w = np.asarray(inputs["lin1_w"], dtype=np.float32)
    l2w = np.asarray(inputs["lin2_w"], dtype=np.float32)
    l2b = np.asarray(inputs["lin2_b"], dtype=np.float32)
    l3w = np.asarray(inputs["lin3_w"], dtype=np.float32)
    l3b = np.asarray(inputs["lin3_b"], dtype=np.float32)

    mw1aug = np.zeros((NINT, HID + 1, HID), dtype=np.float32)
    mw1aug[:, :NGAUSS, :] = mw1
    mw1aug[:, HID, :] = mb1
    mw2x = 0.5 * mw2
    mw2hi = mw2x.astype(np.float16)
    mw2lo = (mw2x - mw2hi.astype(np.float32)).astype(np.float16)
    mb2c = (0.5 * (mb2 - LOG2 * mw2.sum(axis=1))).astype(np.float32)
    l2waug = np.ascontiguousarray(l2w)
    l3waug = np.ascontiguousarray(l3w)
    l2bc = np.ascontiguousarray(l2b.astype(np.float32))          # [NINT, HID]
    l3bc = np.ascontiguousarray(
        (l3b - LOG2 * l3w.sum(axis=1)).astype(np.float32))       # [NINT, HID]

    iota128 = np.broadcast_to(np.arange(128, dtype=np.float32), (128, 128)).copy()
    iota100 = np.arange(100, dtype=np.float32).reshape(100, 1)
    negoffs = np.full((HID, 1), -1.0e4, dtype=np.float32)
    negoffs[:NGAUSS, 0] = -offset

    shared = {
        "emb": emb,
        "mw1aug": mw1aug,
        "mw2hi": mw2hi,
        "mw2lo": mw2lo,
        "mb2c": mb2c,
        "l1w": np.ascontiguousarray(l1w),
        "l2waug": l2waug,
        "l3waug": l3waug,
        "l2bc": l2bc,
        "l3bc": l3bc,
        "iota128": iota128,
        "iota100": iota100,
        "negoffs": negoffs,
    }

    per_core = [dict(shared) for _ in range(NCORES)]
    for (tag, z, pos, edge, cores) in mols:
        pospad = np.zeros((NPAD, 64), dtype=np.float32)
        pospad[:N, :3] = pos
        for c in range(NCORES):
            s_c, d_c, w_c, cnt = cores[c]
            src_pad = np.full(NBLK * 128, N, dtype=np.int64)
            dst_pad = np.full(NBLK * 128, N, dtype=np.int64)
            rel_pad = np.zeros(NBLK * 128, dtype=np.float32)
            off = np.concatenate([[0], np.cumsum(cnt)]).astype(np.int64)
            for w in range(WPC):
                seg = slice(off[w], off[w + 1])
                n = int(off[w + 1] - off[w])
                base = w * BPW * 128
                src_pad[base:base + n] = s_c[seg]
                dst_pad[base:base + n] = d_c[seg] + c * APC
                rel_pad[base:base + n] = (d_c[seg] - w * 128).astype(np.float32)
            m = per_core[c]
            m["srcidx" + tag] = _gather_layout(src_pad)
            m["pdst" + tag] = _gather_layout(dst_pad)
            m["dstrel" + tag] = np.ascontiguousarray(
                rel_pad.reshape(NBLK, 128).T.astype(np.float32))
            m["z" + tag] = np.asarray(
                z[c * APC:(c + 1) * APC], dtype=np.float32).reshape(1, APC)
            m["pospad" + tag] = pospad
    meta = {"BPW": BPW, "NBLK": NBLK, "coeff": coeff}
    return per_core, meta


# ---------------------------------------------------------------------------
# device program
# ---------------------------------------------------------------------------

def build_program(cfg, NBLK, BPW, coeff, use_collective=True):
    N, APC, WPC, NPAD = cfg.N, cfg.APC, cfg.WPC, cfg.NPAD
    NCHUNK = NBLK // 8
    NT = (NBLK + 127) // 128  # d-transpose tiles

    nc = bacc.Bacc("TRN2", num_swdge_queues=4)

    # ---- I/O ----
    ins = {}
    for tag in ("A", "G"):
        ins["srcidx" + tag] = nc.declare_dram_parameter(
            "srcidx" + tag, [128, NBLK * 8], I16, isOutput=False)
        ins["pdst" + tag] = nc.declare_dram_parameter(
            "pdst" + tag, [128, NBLK * 8], I16, isOutput=False)
        ins["dstrel" + tag] = nc.declare_dram_parameter(
            "dstrel" + tag, [128, NBLK], F32, isOutput=False)
        ins["z" + tag] = nc.declare_dram_parameter("z" + tag, [1, APC], F32, isOutput=False)
        ins["pospad" + tag] = nc.declare_dram_parameter(
            "pospad" + tag, [NPAD, 64], F32, isOutput=False)
    ins["emb"] = nc.declare_dram_parameter("emb", [100, 64], F32, isOutput=False)
    ins["mw1aug"] = nc.declare_dram_parameter(
        "mw1aug", [NINT, HID + 1, HID], F32, isOutput=False)
    ins["mw2hi"] = nc.declare_dram_parameter("mw2hi", [NINT, HID, HID], F16, isOutput=False)
    ins["mw2lo"] = nc.declare_dram_parameter("mw2lo", [NINT, HID, HID], F16, isOutput=False)
    ins["mb2c"] = nc.declare_dram_parameter("mb2c", [NINT, HID], F32, isOutput=False)
    ins["l1w"] = nc.declare_dram_parameter("l1w", [NINT, HID, HID], F32, isOutput=False)
    ins["l2waug"] = nc.declare_dram_parameter("l2waug", [NINT, HID, HID], F32, isOutput=False)
    ins["l3waug"] = nc.declare_dram_parameter("l3waug", [NINT, HID, HID], F32, isOutput=False)
    ins["l2bc"] = nc.declare_dram_parameter("l2bc", [NINT, HID], F32, isOutput=False)
    ins["l3bc"] = nc.declare_dram_parameter("l3bc", [NINT, HID], F32, isOutput=False)
    ins["iota128"] = nc.declare_dram_parameter("iota128", [128, 128], F32, isOutput=False)
    ins["iota100"] = nc.declare_dram_parameter("iota100", [100, 1], F32, isOutput=False)
    ins["negoffs"] = nc.declare_dram_parameter("negoffs", [HID, 1], F32, isOutput=False)
    out_dram = nc.declare_dram_parameter("out", [2, 64, 1], F32, isOutput=True)

    # ---- internal DRAM ----
    W_dram = [[nc.dram_tensor(f"W{m}{i}", [128, NBLK * 64], F16)
               for i in range(NINT)] for m in range(2)]
    xshard = [nc.dram_tensor(f"xshard{m}", [APC, 64], F32) for m in range(2)]
    xtab = [nc.dram_tensor(f"xtab{m}", [NPAD, 64], F32, addr_space="Shared")
            for m in range(2)]

    with tile.TileContext(nc) as tc:
        nc.gpsimd.load_library(library_config.mlp)

        cpool = tc.alloc_tile_pool(name="consts", bufs=1)
        ppool = tc.alloc_tile_pool(name="persist", bufs=1)
        spool = tc.alloc_tile_pool(name="stream", bufs=3)
        ohpool = tc.alloc_tile_pool(name="oh", bufs=4)
        eapool = tc.alloc_tile_pool(name="ea", bufs=3)
        bigpool = tc.alloc_tile_pool(name="big", bufs=2)
        pmisc = tc.alloc_tile_pool(name="pmisc", bufs=4, space="PSUM")
        pagg = tc.alloc_tile_pool(name="pagg", bufs=2, space="PSUM")
        pnode = tc.alloc_tile_pool(name="pnode", bufs=2, space="PSUM")

        # ---- constants to SBUF ----
        def cload(name, shape, dtype, src_ap):
            t = cpool.tile(shape, dtype, tag=name, name=name)
            nc.sync.dma_start(out=t[:], in_=src_ap)
            return t

        ident = cpool.tile([128, 128], F32, tag="ident")
        make_identity(nc, ident[:])
        iota128 = cload("iota128", [128, 128], F32, ins["iota128"][:])
        iota100 = cload("iota100", [100, 1], F32, ins["iota100"][:])
        negoffs = cload("negoffs", [HID, 1], F32, ins["negoffs"][:])
        ones64 = cpool.tile([128, HID], F32, tag="ones64")
        nc.vector.memset(ones64[:], 1.0)
        emb = cload("emb", [100, 64], F32, ins["emb"][:])
        mw1aug = cload("mw1aug", [HID + 1, NINT, HID], F32,
                       ins["mw1aug"][:].rearrange("i k m -> k i m"))
        mw2hi = cpool.tile([128, NINT, HID], F16, tag="mw2hi")
        mw2lo = cpool.tile([128, NINT, HID], F16, tag="mw2lo")
        for base in (0, 64):
            nc.sync.dma_start(out=mw2hi[base:base + 64, :, :],
                              in_=ins["mw2hi"][:].rearrange("i k m -> k i m"))
            nc.sync.dma_start(out=mw2lo[base:base + 64, :, :],
                              in_=ins["mw2lo"][:].rearrange("i k m -> k i m"))
        mb2c = []
        for i in range(NINT):
            t = cpool.tile([128, HID], F32, tag=f"mb2c{i}", name=f"mb2c{i}")
            nc.sync.dma_start(
                out=t[:], in_=ins["mb2c"][i:i + 1, :].to_broadcast((128, HID)))
            mb2c.append(t)
        l1w = cload("l1w", [HID, NINT, HID], F32,
                    ins["l1w"][:].rearrange("i k m -> k i m"))
        l2waug = cload("l2waug", [HID, NINT, HID], F32,
                       ins["l2waug"][:].rearrange("i k m -> k i m"))
        l3waug = cload("l3waug", [HID, NINT, HID], F32,
                       ins["l3waug"][:].rearrange("i k m -> k i m"))
        l2bc = cload("l2bc", [HID, NINT], F32,
                     ins["l2bc"][:].rearrange("i k -> k i"))
        l3bc = cload("l3bc", [HID, NINT], F32,
                     ins["l3bc"][:].rearrange("i k -> k i"))
        zerot = cpool.tile([8, 64], F32, tag="zerot")
        nc.vector.memset(zerot[:], 0)
        halfpi = cpool.tile([128, 1], F32, tag="halfpi")
        nc.vector.memset(halfpi[:], math.pi / 2)

        # persistent per-molecule tiles
        hshT = [ppool.tile([64, APC], F32, tag=f"hshT{m}", name=f"hshT{m}") for m in range(2)]
        srcidx = [ppool.tile([128, NBLK * 8], I16, tag=f"srcidx{m}",
                            name=f"srcidx{m}") for m in range(2)]
        dstrel = [ppool.tile([128, NBLK], F32, tag=f"dstrel{m}",
                            name=f"dstrelt{m}") for m in range(2)]
        Cp = [ppool.tile([128, NBLK], F32, tag=f"Cp{m}", name=f"Cp{m}") for m in range(2)]
        d_allm = [ppool.tile([128, NBLK], F32, tag=f"d_all{m}",
                             name=f"d_all{m}") for m in range(2)]

        TAGS = ("A", "G")

        def mol_setup(m):
            tag = TAGS[m]
            nc.sync.dma_start(out=srcidx[m][:], in_=ins["srcidx" + tag][:])
            nc.sync.dma_start(out=dstrel[m][:], in_=ins["dstrel" + tag][:])
            pidx = ppool.tile([128, NBLK * 8], I16, tag="pdstidx")
            nc.sync.dma_start(out=pidx[:], in_=ins["pdst" + tag][:])
            d2_all = ppool.tile([128, NBLK], F32, tag="d2_all")
            # geometry: d^2 per edge (DVE only)
            for c in range(NCHUNK):
                gxs = spool.tile([128, 8, 64], F32, tag="gxs")
                gxd = spool.tile([128, 8, 64], F32, tag="gxd")
                if "no_posgather" in ABLATE:
                    if c == 0:
                        nc.vector.memset(gxs[:], 1.0)
                        nc.vector.memset(gxd[:], 0.5)
                else:
                    nc.gpsimd.dma_gather(
                        gxs[:], ins["pospad" + tag][:],
                        srcidx[m][:, c * 64:(c + 1) * 64], 1024, 1024, 64,
                        queue_num=(2 * c) % 4)
                    nc.gpsimd.dma_gather(
                        gxd[:], ins["pospad" + tag][:],
                        pidx[:, c * 64:(c + 1) * 64], 1024, 1024, 64,
                        queue_num=(2 * c + 1) % 4)
                for s in range(8):
                    B = c * 8 + s
                    df = spool.tile([128, 4], F32, tag="df")
                    nc.vector.tensor_sub(df[:], gxs[:, s, 0:4], gxd[:, s, 0:4])
                    nc.vector.tensor_mul(df[:], df[:], df[:])
                    nc.vector.reduce_sum(d2_all[:, B:B + 1], df[:],
                                         axis=mybir.AxisListType.X)
            # one sqrt + cutoff pass per molecule (batched; keeps ACT tables
            # from thrashing between sqrt/trig/ln_exp sets)
            nc.scalar.activation(d_allm[m][:], d2_all[:],
                                 mybir.ActivationFunctionType.Sqrt)
            sall = ppool.tile([128, NBLK], F32, tag="sall")
            nc.scalar.activation(sall[:], d_allm[m][:],
                                 mybir.ActivationFunctionType.Sin,
                                 scale=-math.pi / CUTOFF, bias=halfpi[:])
            nc.scalar.activation(Cp[m][:], sall[:],
                                 mybir.ActivationFunctionType.Identity,
                                 bias=1.0)

        def h0_phase(m):
            tag = TAGS[m]
            # h0 = emb[z]
            zbc = ppool.tile([100, APC], F32, tag="zbc")
            nc.sync.dma_start(out=zbc[:],
                              in_=ins["z" + tag][:].to_broadcast((100, APC)))
            for t in range(WPC):
                ohz = spool.tile([100, 128], F32, tag="ohz")
                nc.vector.tensor_tensor(
                    ohz[:], zbc[:, t * 128:(t + 1) * 128],
                    iota100[:].to_broadcast((100, 128)),
                    op=mybir.AluOpType.is_equal)
                ph = pmisc.tile([64, 128], F32, tag="pm")
                nc.tensor.matmul(ph[:], emb[:], ohz[:], start=True, stop=True)
                nc.scalar.activation(hshT[m][:, t * 128:(t + 1) * 128], ph[:],
                                     mybir.ActivationFunctionType.Copy)

        def w_production(m):
            wsbs = [None] * NINT
            for qb in range(NBLK // 4):
                B0 = qb * 4
                diag4 = spool.tile([128, 4, 128], F32, tag="diag4", bufs=2)
                nc.gpsimd.affine_select(
                    diag4[:],
                    d_allm[m][:, B0:B0 + 4].rearrange("p (b o) -> p b o", o=1)
                    .to_broadcast((128, 4, 128)),
                    pattern=[[0, 4], [-1, 128]],
                    compare_op=mybir.AluOpType.is_equal,
                    fill=0.0, base=0, channel_multiplier=1)
                pd4 = pmisc.tile([HID, 512], F32, tag="pm", name="pd4")
                nc.tensor.matmul(pd4[:], ones64[:],
                                 diag4[:].rearrange("p b j -> p (b j)"),
                                 start=True, stop=True)
                sq4 = spool.tile([HID, 512], F32, tag="sq4", bufs=2)
                nc.scalar.activation(sq4[:], pd4[:],
                                     mybir.ActivationFunctionType.Square,
                                     bias=negoffs[:])
                ea4 = eapool.tile([HID + 1, 512], F32, tag="ea", name="ea4")
                nc.scalar.activation(ea4[:HID, :], sq4[:],
                                     mybir.ActivationFunctionType.Exp,
                                     scale=coeff)
                nc.vector.memset(ea4[64:65, :], 1.0)
                for sb in range(4):
                    B = B0 + sb
                    c, cs = divmod(B, 8)
                    if cs == 0:
                        for i in range(NINT):
                            wsbs[i] = spool.tile([128, 8, 64], F16,
                                                 tag=f"wsb{i}", bufs=2,
                                                 name=f"wsb{i}")
                    ea_s = ea4[:, sb * 128:(sb + 1) * 128]
                    ps6 = pmisc.tile([128, 384], F32, tag="pm", name="ps6")
                    for g in range(2):
                        for j in range(3):
                            i = g * 3 + j
                            nc.tensor.matmul(
                                ps6[g * 64:(g + 1) * 64,
                                    j * 128:(j + 1) * 128],
                                mw1aug[:, i, :], ea_s,
                                start=True, stop=True)
                    ex6 = spool.tile([128, 384], F32, tag="ex3", bufs=2)
                    nc.scalar.activation(ex6[:], ps6[:],
                                         mybir.ActivationFunctionType.Exp)
                    ssp6 = spool.tile([128, 384], F16, tag="ssp3", bufs=2)
                    nc.scalar.activation(ssp6[:], ex6[:],
                                         mybir.ActivationFunctionType.Ln,
                                         bias=1.0)
                    for g in range(2):
                        for j in range(3):
                            i = g * 3 + j
                            lhs = ssp6[g * 64:(g + 1) * 64,
                                       j * 128:(j + 1) * 128]
                            pw = pmisc.tile([128, 64], F32, tag="pm",
                                            name="pw")
                            nc.tensor.matmul(
                                pw[:], lhs,
                                mw2hi[g * 64:(g + 1) * 64, i, :],
                                start=True, stop=False)
                            nc.tensor.matmul(
                                pw[:], lhs,
                                mw2lo[g * 64:(g + 1) * 64, i, :],
                                start=False, stop=True)
                            wb = spool.tile([128, 64], F32, tag="wb")
                            nc.vector.tensor_add(wb[:], pw[:], mb2c[i][:])
                            nc.vector.tensor_mul(
                                wsbs[i][:, cs, :], wb[:],
                                Cp[m][:, B:B + 1].to_broadcast((128, 64)))
                    if cs == 7:
                        for i in range(NINT):
                            nc.sync.dma_start(
                                out=W_dram[m][i][:, c * 512:(c + 1) * 512],
                                in_=wsbs[i][:])

        def x_phase(m, i):
            xshT = bigpool.tile([64, APC], F32, tag="xshT")
            for q0 in range(0, APC, 512):
                qn = min(512, APC - q0)
                sl = slice(q0, q0 + qn)
                px = pnode.tile([64, 512], F32, tag="pnode")
                nc.tensor.matmul(px[:, :qn], l1w[:, i, :],
                                 hshT[m][:, sl], start=True, stop=True)
                nc.scalar.activation(xshT[:, sl], px[:, :qn],
                                     mybir.ActivationFunctionType.Copy)
            xsh = bigpool.tile([128, WPC, 64], F32, tag="xsh")
            for t in range(WPC):
                ptr = pmisc.tile([128, 128], F32, tag="pm")
                nc.tensor.transpose(ptr[:, :64],
                                    xshT[:, t * 128:(t + 1) * 128],
                                    ident[:64, :64])
                nc.vector.tensor_copy(xsh[:, t, :], ptr[:, :64])
            nc.sync.dma_start(
                out=xshard[m][:].rearrange("(t p) f -> p t f", p=128),
                in_=xsh[:])
            if i == 0:
                nc.sync.dma_start(out=xtab[m][N:NPAD, :],
                                  in_=zerot[:NPAD - N, :])
            if use_collective:
                nc.gpsimd.collective_compute(
                    "AllGather", mybir.AluOpType.bypass,
                    replica_groups=[list(range(NCORES))],
                    ins=[xshard[m][:]],
                    outs=[xtab[m][0:N, :]])
            else:
                nc.sync.dma_start(out=xtab[m][0:APC, :], in_=xshard[m][:])

        def edge_phase(m, i):
            aggT = bigpool.tile([HID, APC], F32, tag="aggT")
            for w in range(WPC):
                pg = pagg.tile([64, 128], F32, tag="pagg")
                for blk in range(BPW):
                    B = w * BPW + blk
                    c, s = divmod(B, 8)
                    if s == 0:
                        gx = spool.tile([128, 8, 64], F32, tag="gx")
                        if "gather_seq" in ABLATE:
                            nc.sync.dma_start(
                                out=gx[:],
                                in_=xtab[m][0:1024, :].rearrange(
                                    "(b p) f -> p b f", p=128))
                        else:
                            nc.gpsimd.dma_gather(
                                gx[:], xtab[m][:],
                                srcidx[m][:, c * 64:(c + 1) * 64], 1024, 1024, 64,
                                queue_num=c % 4)
                        wt = spool.tile([128, 8, 64], F16, tag="wt")
                        nc.sync.dma_start(
                            out=wt[:],
                            in_=W_dram[m][i][:, c * 512:(c + 1) * 512])
                    if "no_oh" in ABLATE:
                        if "no_scatter" not in ABLATE:
                            nc.tensor.matmul(pg[:], gx[:, s, :], iota128[:],
                                             start=(blk == 0),
                                             stop=(blk == BPW - 1))
                    else:
                        oh = ohpool.tile([128, 128], F16, tag="oh")
                        nc.vector.tensor_tensor(
                            oh[:], dstrel[m][:, B:B + 1].to_broadcast((128, 128)),
                            iota128[:], op=mybir.AluOpType.is_equal)
                        msg = ohpool.tile([128, 64], F16, tag="msg")
                        nc.vector.tensor_mul(msg[:], gx[:, s, :], wt[:, s, :])
                        if "no_scatter" not in ABLATE:
                            nc.tensor.matmul(pg[:], msg[:], oh[:],
                                             start=(blk == 0),
                                             stop=(blk == BPW - 1))
                if "no_scatter" in ABLATE:
                    nc.vector.memset(aggT[:HID, w * 128:(w + 1) * 128], 0.0)
                else:
                    nc.scalar.activation(
                        aggT[:HID, w * 128:(w + 1) * 128], pg[:],
                        mybir.ActivationFunctionType.Copy)
            # node MLP: h += (ssp(agg@l2w+l2b))@l3w + l3b
            saugT = bigpool.tile([HID, APC], F32, tag="saugT")
            for q0 in range(0, APC, 512):
                qn = min(512, APC - q0)
                sl = slice(q0, q0 + qn)
                pz = pnode.tile([64, 512], F32, tag="pnode")
                nc.tensor.matmul(pz[:, :qn], l2waug[:, i, :],
                                 aggT[:, sl], start=True, stop=True)
                ez = spool.tile([64, 512], F32, tag="ez")
                nc.scalar.activation(ez[:, :qn], pz[:, :qn],
                                     mybir.ActivationFunctionType.Exp,
                                     bias=l2bc[:, i:i + 1])
                nc.scalar.activation(saugT[:HID, sl], ez[:, :qn],
                                     mybir.ActivationFunctionType.Ln, bias=1.0)
            for q0 in range(0, APC, 512):
                qn = min(512, APC - q0)
                sl = slice(q0, q0 + qn)
                px2 = pnode.tile([64, 512], F32, tag="pnode")
                nc.tensor.matmul(px2[:, :qn], l3waug[:, i, :],
                                 saugT[:, sl], start=True, stop=True)
                nc.vector.scalar_tensor_tensor(
                    out=hshT[m][:, sl], in0=px2[:, :qn],
                    scalar=l3bc[:, i:i + 1], in1=hshT[m][:, sl],
                    op0=mybir.AluOpType.add, op1=mybir.AluOpType.add)

        # ---- schedule ----
        for m in range(2):
            mol_setup(m)
        for m in range(2):
            h0_phase(m)
            x_phase(m, 0)
        if "wprod" not in ABLATE:
            for m in range(2):
                w_production(m)
        for i in range(NINT):
            for m in range(2):
                edge_phase(m, i)
                if i < NINT - 1:
                    x_phase(m, i + 1)
        for m in range(2):
            rsum = spool.tile([64, 1], F32, tag="rsum")
            nc.vector.reduce_sum(rsum[:], hshT[m][:],
                                 axis=mybir.AxisListType.X)
            nc.sync.dma_start(out=out_dram[m, :, :], in_=rsum[:])

        for p in (pnode, pagg, pmisc, bigpool, eapool, ohpool, spool,
                  ppool, cpool):
            p.release()

    nc.compile()
    return nc


# ---------------------------------------------------------------------------
# host entry
# ---------------------------------------------------------------------------

_prog_cache = {}


def _run(inputs, cfg, trace=False):
    in_maps, meta = prep_inputs(inputs, cfg)
    key = (cfg.N, cfg.E, meta["BPW"])
    if key not in _prog_cache:
        _prog_cache[key] = build_program(cfg, meta["NBLK"], meta["BPW"],
                                         meta["coeff"])
    nc = _prog_cache[key]
    res = run_bass_kernel_spmd(nc, in_maps, core_ids=list(range(NCORES)),
                               trace=trace)
    return res


def head_host(eA, eG, inputs):
    add = np.asarray(inputs["add_features"], dtype=np.float32)
    fc1_w = np.asarray(inputs["fc1_w"], dtype=np.float32)
    fc1_b = np.asarray(inputs["fc1_b"], dtype=np.float32)
    fc2_w = np.asarray(inputs["fc2_w"], dtype=np.float32)
    fc2_b = np.asarray(inputs["fc2_b"], dtype=np.float32)
    alpha = np.float32(np.asarray(inputs["prelu_a"]))
    pool = np.concatenate([eA, eG, add]).astype(np.float32)
    x = pool @ fc1_w + fc1_b
    x = np.where(x >= 0, x, alpha * x)
    x = x @ fc2_w + fc2_b
    return np.exp(x).astype(np.float32)


def kernel(**inputs):
    cfg = Cfg(N=16384, E=524288, NGRAPHS=256)
    res = _run(inputs, cfg)
    sums = np.zeros((2, 64), dtype=np.float64)
    for r in res.results:
        sums += r["out"][:, :, 0].astype(np.float64)
    eA = (sums[0] / cfg.NGRAPHS).astype(np.float32)
    eG = (sums[1] / cfg.NGRAPHS).astype(np.float32)
    return head_host(eA, eG, inputs)

